# revision 3
# baseline (speedup 1.0000x reference)
"""Trainium2 Bass kernel for CRPNet head (nms_detection problem).

Workload: 5 FPN feature levels x 2 images, three conv subnets (cls/bbox/kpt),
each 4x [conv3x3(256->256) + bias + relu] followed by a conv3x3 head
(cls: 720ch, bbox: 36ch, kpt: 306ch), flattened to per-anchor predictions,
plus the Box2KptTransform keypoint decode.

Strategy (8 NeuronCores, SPMD single program with a partition_id branch):
  - cores 0..5 (branch A): one full (image, subnet) level-0 80x80 map each.
  - cores 6..7 (branch B): all level 1-4 maps for one image, all 3 subnets.
Convs are emulated as 9 accumulated shifted matmuls over zero-separated
"canvas" rects in SBUF (SAME padding comes from the zero borders, which stay
zero because separators are never written). All conv matmuls are
channel-major (weights stationary, one free dim; shifted canvas windows are
the moving operand). Head outputs and the keypoint decode stay channel-major
on device; the host transposes to row-major when scattering into the final
outputs. Matmul operands are fp16 (PSUM accumulates fp32); measured
per-matmul relative error ~3e-4.
"""

import os
import sys
import types

import numpy as np

# ----------------------------------------------------------------- constants
P = 128
HC = WC = 86  # canvas incl. 1-cell zero border
LEVEL_HW = [80, 40, 20, 10, 5]
N_IMG = 2
A = 9
KPT = 17
C_CLS = 80
NBLK = 6  # head cout blocks of 128 (6*128 = 768 >= 720)
TOTPOS = 6400  # per-core position slots (A: 6400, B: 6375)

SUBNETS = ("cls", "bbox", "kpt")
HEAD_CH = {"cls": 720, "bbox": 36, "kpt": 306}
HEAD_BLOCKS = {"cls": 6, "bbox": 1, "kpt": 3}

_HWA = [A * h * h for h in LEVEL_HW]
_OFF = [0]
for _s in _HWA:
    _OFF.append(_OFF[-1] + _s)
ROWS_PER_IMG = _OFF[-1]  # 76725

# rects: (r0, c0, h, w, slot, level) in canvas coords (interior starts at 1)
RECTS_A = [(1, 1, 80, 80, 0, 0)]
RECTS_B = [
    (1, 1, 40, 40, 0, 1), (1, 42, 40, 40, 1, 1), (42, 1, 40, 40, 2, 1),
    (42, 42, 20, 20, 0, 2), (42, 63, 20, 20, 1, 2), (63, 42, 20, 20, 2, 2),
    (63, 63, 10, 10, 0, 3), (63, 74, 10, 10, 1, 3), (74, 63, 10, 10, 2, 3),
    (74, 74, 5, 5, 0, 4), (74, 80, 5, 5, 1, 4), (80, 74, 5, 5, 2, 4),
]


def rect_pos_bases(rects):
    """Contiguous per-rect position ranges inside [0, TOTPOS)."""
    bases = []
    off = 0
    for (_r0, _c0, h, w, _s, _l) in rects:
        bases.append(off)
        off += h * w
    assert off <= TOTPOS
    return bases


POS_BASE_A = rect_pos_bases(RECTS_A)
POS_BASE_B = rect_pos_bases(RECTS_B)


def trunk_tiles(h, w):
    """Row-strips of <=512 cells: list of (row_start, n_rows)."""
    rpt = min(h, 512 // w)
    out = []
    r = 0
    while r < h:
        n = min(rpt, h - r)
        out.append((r, n))
        r += n
    return out


# ------------------------------------------------------------- program build
_PROG_CACHE = {}


def _build_program():
    from concourse import bacc, mybir
    import concourse.tile as tile

    F16 = mybir.dt.float16
    F32 = mybir.dt.float32

    nc = bacc.Bacc(
        "TRN2",
        target_bir_lowering=False,
        debug=False,
        enable_asserts=False,
        num_devices=8,
    )

    feat = nc.dram_tensor("feat", [2, P, 2, HC, WC], F16, kind="ExternalInput").ap()
    tw = nc.dram_tensor("tw", [4, P, 3, 2, 2, 9, P], F16, kind="ExternalInput").ap()
    tb = nc.dram_tensor("tb", [P, 24], F32, kind="ExternalInput").ap()
    hw = nc.dram_tensor("hw", [3, P, 2, NBLK, 9, P], F16, kind="ExternalInput").ap()
    hb = nc.dram_tensor("hb", [P, 3, NBLK], F32, kind="ExternalInput").ap()
    whm = nc.dram_tensor("whm", [3, P, TOTPOS], F32, kind="ExternalInput").ap()
    cxm = nc.dram_tensor("cxm", [3, P, TOTPOS], F32, kind="ExternalInput").ap()
    head_out = nc.dram_tensor(
        "head_out", [NBLK, P, TOTPOS], F32, kind="ExternalOutput").ap()
    kpts_out = nc.dram_tensor(
        "kpts_out", [3, P, TOTPOS], F32, kind="ExternalOutput").ap()

    with tile.TileContext(nc) as tc:
        with (
            tc.tile_pool(name="canv", bufs=1) as canv_pool,
            tc.tile_pool(name="tw", bufs=2) as tw_pool,
            tc.tile_pool(name="hw", bufs=2) as hw_pool,
            tc.tile_pool(name="const", bufs=1) as const_pool,
            tc.tile_pool(name="st", bufs=4) as st_pool,
            tc.tile_pool(name="kst", bufs=3) as kst_pool,
            tc.tile_pool(name="whx", bufs=3) as whx_pool,
            tc.tile_pool(name="tps", bufs=4, space="PSUM") as tps_pool,
            tc.tile_pool(name="hps", bufs=4, space="PSUM") as hps_pool,
        ):
            cv0 = canv_pool.tile([P, 2, HC, WC], F16, tag="cv0")
            cv1 = canv_pool.tile([P, 2, HC, WC], F16, tag="cv1")
            tbt = const_pool.tile([P, 24], F32, tag="tb")
            hbt = const_pool.tile([P, 3, NBLK], F32, tag="hb")
            nc.sync.dma_start(cv0[:], feat[0])
            nc.sync.dma_start(cv1[:], feat[1])
            nc.sync.dma_start(tbt[:], tb)
            nc.sync.dma_start(hbt[:], hb)

            def emit_conv_layer(rects, layer, cv_in, cv_out, twt):
                for (r0, c0, h, w, slot, _lvl) in rects:
                    for cob in range(2):
                        bias = tbt[:, slot * 8 + layer * 2 + cob][:, None]
                        for (tr, nr) in trunk_tiles(h, w):
                            ps = tps_pool.tile([P, 480], F32, tag="tps")
                            ps = ps[:, : nr * w]
                            k = 0
                            for dy in (-1, 0, 1):
                                for dx in (-1, 0, 1):
                                    for cib in range(2):
                                        rhs = cv_in[
                                            :, cib,
                                            r0 + tr + dy: r0 + tr + dy + nr,
                                            c0 + dx: c0 + dx + w,
                                        ]
                                        nc.tensor.matmul(
                                            ps,
                                            twt[:, slot, cib, cob, k, :],
                                            rhs,
                                            start=(k == 0 and cib == 0),
                                            stop=(k == 8 and cib == 1),
                                        )
                                    k += 1
                            nc.scalar.activation(
                                cv_out[:, cob, r0 + tr: r0 + tr + nr, c0: c0 + w],
                                ps,
                                mybir.ActivationFunctionType.Relu,
                                bias=bias,
                            )

            def emit_head_rect(rect, pos_base, cv, hwt, nblocks, decode):
                (r0, c0, h, w, slot, _lvl) = rect
                for (tr, nr) in trunk_tiles(h, w):
                    p0 = pos_base + tr * w
                    n = nr * w
                    for cb in range(nblocks):
                        ps = hps_pool.tile([P, 480], F32, tag="hps")
                        ps = ps[:, :n]
                        k = 0
                        for dy in (-1, 0, 1):
                            for dx in (-1, 0, 1):
                                for cib in range(2):
                                    rhs = cv[
                                        :, cib,
                                        r0 + tr + dy: r0 + tr + dy + nr,
                                        c0 + dx: c0 + dx + w,
                                    ]
                                    nc.tensor.matmul(
                                        ps,
                                        hwt[:, cib, cb, k, :],
                                        rhs,
                                        start=(k == 0 and cib == 0),
                                        stop=(k == 8 and cib == 1),
                                    )
                                k += 1
                        st = st_pool.tile([P, 480], F32, tag="st")
                        st = st[:, :n]
                        nc.scalar.activation(
                            st,
                            ps,
                            mybir.ActivationFunctionType.Identity,
                            bias=hbt[:, slot, cb][:, None],
                        )
                        nc.sync.dma_start(head_out[cb, :, p0: p0 + n], st)
                        if decode and cb < 3:
                            wht = whx_pool.tile([P, 480], F32, tag="wh")
                            cxt = whx_pool.tile([P, 480], F32, tag="cx")
                            nc.sync.dma_start(wht[:, :n], whm[cb, :, p0: p0 + n])
                            nc.sync.dma_start(cxt[:, :n], cxm[cb, :, p0: p0 + n])
                            kst = kst_pool.tile([P, 480], F32, tag="kst")
                            kst = kst[:, :n]
                            nc.vector.tensor_tensor(
                                kst, st, wht[:, :n], mybir.AluOpType.mult)
                            nc.vector.tensor_tensor(
                                kst, kst, cxt[:, :n], mybir.AluOpType.add)
                            nc.sync.dma_start(kpts_out[cb, :, p0: p0 + n], kst)

            def emit_branch(branch):
                rects = RECTS_A if branch == "A" else RECTS_B
                bases = POS_BASE_A if branch == "A" else POS_BASE_B
                nslot = 1 if branch == "A" else 3
                cvs = [cv0, cv1]
                for layer in range(4):
                    twt = tw_pool.tile([P, 3, 2, 2, 9, P], F16, tag="tw")
                    if nslot == 1:
                        nc.sync.dma_start(twt[:, 0:1], tw[layer][:, 0:1])
                    else:
                        nc.sync.dma_start(twt[:], tw[layer])
                    emit_conv_layer(rects, layer, cvs[layer % 2],
                                    cvs[(layer + 1) % 2], twt)
                for g in range(nslot):
                    hwt = hw_pool.tile([P, 2, NBLK, 9, P], F16, tag="hw")
                    nc.sync.dma_start(hwt[:], hw[g])
                    if branch == "A":
                        nblocks, decode = NBLK, True
                    else:
                        nblocks = HEAD_BLOCKS[SUBNETS[g]]
                        decode = SUBNETS[g] == "kpt"
                    for ri, rect in enumerate(rects):
                        if rect[4] != g:
                            continue
                        emit_head_rect(rect, bases[ri], cv0, hwt, nblocks, decode)

            pid = nc.partition_id()
            with tc.If(pid < 6) as cmp:
                emit_branch("A")
            with cmp.Else():
                emit_branch("B")

    nc.compile()
    return nc


# ------------------------------------------------------------ host-side prep
def _pack_trunk_w(w4):
    """(4,256,256,3,3) f32 -> (4,128,4608) f16 lhsT slab (cib,cob,tap,coi)."""
    y = w4.reshape(4, 2, P, 2, P, 3, 3)  # l, cob, coi, cib, cii, dy, dx
    x = np.transpose(y, (0, 4, 3, 1, 5, 6, 2))  # l, cii, cib, cob, dy, dx, coi
    return np.ascontiguousarray(x.reshape(4, P, 2 * 2 * 9 * P)).astype(np.float16)


def _pack_head_w(wh_raw):
    """(Cout,256,3,3) f32 -> (128, 2, NBLK, 9, 128) f16 lhsT, cout padded."""
    cout = wh_raw.shape[0]
    z = np.zeros((NBLK * P, 256, 3, 3), np.float32)
    z[:cout] = wh_raw
    y = z.reshape(NBLK, P, 2, P, 3, 3).reshape(NBLK, P, 2, P, 9)
    # y[cb, coi, cib, cii, tap] -> (cii, cib, cb, tap, coi)
    t = np.transpose(y, (3, 2, 0, 4, 1))
    return np.ascontiguousarray(t).astype(np.float16)


def _feat_to_canvas_block(f):
    """(256,h,w) f32 -> (128,2,h,w) f16."""
    return np.transpose(
        f.reshape(2, P, f.shape[1], f.shape[2]), (1, 0, 2, 3)
    ).astype(np.float16)


def _core_plan(core):
    if core < 6:
        img, sub = core // 3, core % 3
        return "A", img, RECTS_A, POS_BASE_A, {0: sub}
    img = core - 6
    return "B", img, RECTS_B, POS_BASE_B, {0: 0, 1: 1, 2: 2}


def _pos_rowbase(img, rects, bases):
    """(TOTPOS,) global anchor-row base per position slot (-1 invalid)."""
    rb = np.full(TOTPOS, -1, np.int64)
    for (r0c, c0c, h, w, slot, lvl), base in zip(rects, bases):
        n = h * w
        rb[base: base + n] = img * ROWS_PER_IMG + _OFF[lvl] + np.arange(n) * A
    return rb


def kernel(**inputs):
    inputs = {k: np.asarray(v) for k, v in inputs.items()}
    from concourse import bass_utils
    from concourse.bass_interp import get_hw_module

    if "nc" not in _PROG_CACHE:
        nc = _build_program()
        nc.m = get_hw_module(nc.m)
        _PROG_CACHE["nc"] = nc
    nc = _PROG_CACHE["nc"]

    feats = [inputs[f"feat{i}"] for i in range(5)]
    boxes = inputs["boxes"].astype(np.float32)
    bw = boxes[:, 2] - boxes[:, 0]
    bh = boxes[:, 3] - boxes[:, 1]
    bcx = boxes[:, 0] + 0.5 * bw
    bcy = boxes[:, 1] + 0.5 * bh

    tw_by_sub, hw_by_sub, tb_by_sub, hb_by_sub = {}, {}, {}, {}
    for si, sub in enumerate(SUBNETS):
        wkey = {"cls": "cls_w", "bbox": "bbox_w", "kpt": "kpt_w"}[sub]
        bkey = {"cls": "cls_b", "bbox": "bbox_b", "kpt": "kpt_b"}[sub]
        hkey = {"cls": "cls_score_w", "bbox": "bbox_pred_w", "kpt": "kpt_pred_w"}[sub]
        hbkey = {"cls": "cls_score_b", "bbox": "bbox_pred_b", "kpt": "kpt_pred_b"}[sub]
        tw_by_sub[si] = _pack_trunk_w(inputs[wkey].astype(np.float32))
        hw_by_sub[si] = _pack_head_w(inputs[hkey].astype(np.float32))
        tb_by_sub[si] = inputs[bkey].astype(np.float32)  # (4, 256)
        hbv = np.zeros(NBLK * P, np.float32)
        hbv[: HEAD_CH[sub]] = inputs[hbkey].astype(np.float32)
        hb_by_sub[si] = hbv

    # channel-row decomposition for the kpt decode: channel c = a*34 + k*2 + xy
    c_all = np.arange(3 * P)
    c_a = c_all // 34
    c_xy = c_all % 2
    c_valid = c_all < A * KPT * 2  # 306

    in_maps = []
    rowbases = []
    for core in range(8):
        branch, img, rects, bases, slot_sub = _core_plan(core)

        feat_arr = np.zeros((2, P, 2, HC, WC), np.float16)
        for (r0, c0, h, w, slot, lvl) in rects:
            feat_arr[0, :, :, r0: r0 + h, c0: c0 + w] = _feat_to_canvas_block(
                np.asarray(feats[lvl][img], np.float32))

        tw_arr = np.zeros((4, P, 3, 4608), np.float16)
        tb_arr = np.zeros((P, 24), np.float32)
        hw_arr = np.zeros((3, P, 2, NBLK, 9, P), np.float16)
        hb_arr = np.zeros((P, 3, NBLK), np.float32)
        for s, si in slot_sub.items():
            tw_arr[:, :, s, :] = tw_by_sub[si]
            hw_arr[s] = hw_by_sub[si]
            hb_arr[:, s, :] = hb_by_sub[si].reshape(NBLK, P).T
            for layer in range(4):
                for cob in range(2):
                    tb_arr[:, s * 8 + layer * 2 + cob] = tb_by_sub[si][
                        layer, cob * P: (cob + 1) * P]

        rb = _pos_rowbase(img, rects, bases)
        rowbases.append(rb)
        valid_pos = rb >= 0
        wh_arr = np.zeros((3, P, TOTPOS), np.float32)
        cx_arr = np.zeros((3, P, TOTPOS), np.float32)
        # rows = rb[pos] + a[c]; value = (xy ? bh/bcy : bw/bcx)[row]
        pos_idx = np.where(valid_pos)[0]
        rows = rb[pos_idx][None, :] + c_a[c_valid][:, None]  # (306, npos)
        wh_vals = np.where(c_xy[c_valid][:, None] == 0, bw[rows], bh[rows])
        cx_vals = np.where(c_xy[c_valid][:, None] == 0, bcx[rows], bcy[rows])
        whf = wh_arr.reshape(3 * P, TOTPOS)
        cxf = cx_arr.reshape(3 * P, TOTPOS)
        whf[np.ix_(c_all[c_valid], pos_idx)] = wh_vals
        cxf[np.ix_(c_all[c_valid], pos_idx)] = cx_vals

        in_maps.append({
            "feat": feat_arr,
            "tw": np.ascontiguousarray(tw_arr.reshape(4, P, 3, 2, 2, 9, P)),
            "tb": tb_arr,
            "hw": hw_arr,
            "hb": hb_arr,
            "whm": wh_arr,
            "cxm": cx_arr,
        })

    trace = os.environ.get("CRP_TRACE") == "1"
    kwargs = {}
    if trace:
        _install_ntff_shim()
        kwargs = dict(trace=True, trace_cores=[0, 6])
    res = bass_utils.run_bass_kernel_spmd(
        nc, in_maps, core_ids=list(range(8)), **kwargs)
    globals()["LAST_EXEC_NS"] = res.exec_time_ns

    # ---------------------------------------------------------------- gather
    n_rows = N_IMG * ROWS_PER_IMG
    box_cls = np.zeros((n_rows, C_CLS), np.float32)
    box_delta = np.zeros((n_rows, 4), np.float32)
    pred_kpts = np.zeros((n_rows, KPT, 3), np.float32)
    pred_kpts[:, :, 2] = 1.0

    for core in range(8):
        branch, img, rects, bases, slot_sub = _core_plan(core)
        ho = res.results[core]["head_out"].reshape(NBLK * P, TOTPOS)
        ko = res.results[core]["kpts_out"].reshape(3 * P, TOTPOS)
        rb = rowbases[core]
        for (r0c, c0c, h, w, slot, lvl), base in zip(rects, bases):
            sub = SUBNETS[slot_sub[slot]]
            n = h * w
            sl = slice(base, base + n)
            rows = (rb[sl][:, None] + np.arange(A)[None, :]).ravel()
            if sub == "cls":
                # ho[(a*80+c), pos] -> (pos, a, c)
                arr = ho[:720, sl].reshape(A, C_CLS, n)
                box_cls[rows] = np.transpose(arr, (2, 0, 1)).reshape(n * A, C_CLS)
            elif sub == "bbox":
                arr = ho[:36, sl].reshape(A, 4, n)
                box_delta[rows] = np.transpose(arr, (2, 0, 1)).reshape(n * A, 4)
            else:
                arr = ko[: A * KPT * 2, sl].reshape(A, KPT, 2, n)
                pred_kpts[rows, :, 0:2] = np.transpose(
                    arr, (3, 0, 1, 2)).reshape(n * A, KPT, 2)

    return box_cls, box_delta, pred_kpts


# -------------------------------------------------- optional NTFF trace shim
def _install_ntff_shim():
    try:
        import antenv

        mod = sys.modules.get("antenv.axon_hooks")
        if mod is None or not hasattr(mod, "get_axon_ntff_profile_hook"):
            mod = types.ModuleType("antenv.axon_hooks")
            mod._hook = None
            mod.set_axon_ntff_profile_hook = lambda h: setattr(mod, "_hook", h)
            mod.get_axon_ntff_profile_hook = lambda: mod._hook
            sys.modules["antenv.axon_hooks"] = mod
            antenv.axon_hooks = mod
        from trn_agent_boot.trn_boot import _ntff_profile_via_ctypes

        hook = _ntff_profile_via_ctypes("/opt/axon/libaxon_pjrt.so")
        if hook is not None:
            mod.set_axon_ntff_profile_hook(hook)
    except Exception:
        pass


# revision 5
# speedup vs baseline: 1.0934x; 1.0934x over previous
"""Trainium2 Bass kernel for CRPNet head (nms_detection problem).

Workload: 5 FPN feature levels x 2 images, three conv subnets (cls/bbox/kpt),
each 4x [conv3x3(256->256) + bias + relu] followed by a conv3x3 head
(cls: 720ch, bbox: 36ch, kpt: 306ch), flattened to per-anchor predictions,
plus the Box2KptTransform keypoint decode.

Strategy (8 NeuronCores, SPMD single program, partition_id branches):
  - cores 0..3 (branch A1): cls + bbox half-maps (level-0 rows split in two,
    5-row shrinking halo; bottom halves are vertically flipped on the host —
    flipped features + dy-flipped weights — so both halves share one geometry).
  - cores 4..5 (branch A2): two kpt half-maps (+ keypoint decode).
  - cores 6..7 (branch B): all level 1-4 maps for one image, all 3 subnets.
Convs are emulated as 9 accumulated shifted matmuls over zero-separated
"canvas" rects in SBUF (SAME padding comes from the zero borders, which stay
zero because separators are never written). All conv matmuls are
channel-major (weights stationary, one free dim; shifted canvas windows are
the moving operand — the stationary operand must be 1-D free on walrus).
Head outputs and the keypoint decode stay channel-major on device; the host
transposes when scattering into the final outputs. Matmul operands are fp16
(PSUM accumulates fp32); per-matmul relative error ~3e-4.
"""

import os
import sys
import types

import numpy as np

# ----------------------------------------------------------------- constants
P = 128
CH, CW = 94, 86  # canvas incl. zero borders/separators
LEVEL_HW = [80, 40, 20, 10, 5]
N_IMG = 2
A = 9
KPT = 17
C_CLS = 80
NBLK = 6  # head cout blocks of 128 (6*128 = 768 >= 720)
TOTPOS = 6400  # per-core head position slots (A: 6400, B: 6375)
HALF = 40  # L0 half-map target rows
HIN = 45  # L0 half-map input rows (target + 5 halo)

SUBNETS = ("cls", "bbox", "kpt")
HEAD_CH = {"cls": 720, "bbox": 36, "kpt": 306}
HEAD_BLOCKS = {"cls": 6, "bbox": 1, "kpt": 3}

_HWA = [A * h * h for h in LEVEL_HW]
_OFF = [0]
for _s in _HWA:
    _OFF.append(_OFF[-1] + _s)
ROWS_PER_IMG = _OFF[-1]  # 76725

# rects: (r0, c0, h_in, w, slot, level, shrink)
#   shrink=True: conv layer l writes local rows [0, h_in-1-l); head writes
#   [0, h_in-5). shrink=False: every layer writes all h_in rows.
RECTS_A = [
    (1, 1, HIN, 80, 0, 0, True),
    (47, 1, HIN, 80, 1, 0, True),
]
RECTS_B = [
    (1, 1, 40, 40, 0, 1, False), (1, 42, 40, 40, 1, 1, False),
    (42, 1, 40, 40, 2, 1, False),
    (42, 42, 20, 20, 0, 2, False), (42, 63, 20, 20, 1, 2, False),
    (63, 42, 20, 20, 2, 2, False),
    (63, 63, 10, 10, 0, 3, False), (63, 74, 10, 10, 1, 3, False),
    (74, 63, 10, 10, 2, 3, False),
    (74, 74, 5, 5, 0, 4, False), (74, 80, 5, 5, 1, 4, False),
    (80, 74, 5, 5, 2, 4, False),
]


def rect_head_hw(rect):
    (_r0, _c0, h, w, _s, _l, shrink) = rect
    return (h - 5, w) if shrink else (h, w)


def rect_pos_bases(rects):
    bases, off = [], 0
    for rect in rects:
        bases.append(off)
        hh, ww = rect_head_hw(rect)
        off += hh * ww
    assert off <= TOTPOS
    return bases


POS_BASE_A = rect_pos_bases(RECTS_A)
POS_BASE_B = rect_pos_bases(RECTS_B)


def trunk_tiles(h, w):
    """Row-strips of <=512 cells: list of (row_start, n_rows)."""
    rpt = min(h, 512 // w)
    out = []
    r = 0
    while r < h:
        n = min(rpt, h - r)
        out.append((r, n))
        r += n
    return out


# ------------------------------------------------------------- program build
_PROG_CACHE = {}


def _build_program():
    from concourse import bacc, mybir
    import concourse.tile as tile

    F16 = mybir.dt.float16
    F32 = mybir.dt.float32

    nc = bacc.Bacc(
        "TRN2",
        target_bir_lowering=False,
        debug=False,
        enable_asserts=False,
        num_devices=8,
    )

    feat = nc.dram_tensor("feat", [2, P, 2, CH, CW], F16, kind="ExternalInput").ap()
    tw = nc.dram_tensor("tw", [4, P, 3, 2, 2, 9, P], F16, kind="ExternalInput").ap()
    tb = nc.dram_tensor("tb", [P, 24], F32, kind="ExternalInput").ap()
    hw = nc.dram_tensor("hw", [3, P, 2, NBLK, 9, P], F16, kind="ExternalInput").ap()
    hb = nc.dram_tensor("hb", [P, 3, NBLK], F32, kind="ExternalInput").ap()
    whm = nc.dram_tensor("whm", [3, P, TOTPOS], F32, kind="ExternalInput").ap()
    cxm = nc.dram_tensor("cxm", [3, P, TOTPOS], F32, kind="ExternalInput").ap()
    head_out = nc.dram_tensor(
        "head_out", [NBLK, P, TOTPOS], F32, kind="ExternalOutput").ap()
    kpts_out = nc.dram_tensor(
        "kpts_out", [3, P, TOTPOS], F32, kind="ExternalOutput").ap()

    with tile.TileContext(nc) as tc:
        with (
            tc.tile_pool(name="canv", bufs=1) as canv_pool,
            tc.tile_pool(name="tw", bufs=2) as tw_pool,
            tc.tile_pool(name="hw", bufs=2) as hw_pool,
            tc.tile_pool(name="const", bufs=1) as const_pool,
            tc.tile_pool(name="st", bufs=4) as st_pool,
            tc.tile_pool(name="kst", bufs=3) as kst_pool,
            tc.tile_pool(name="whx", bufs=3) as whx_pool,
            tc.tile_pool(name="tps", bufs=4, space="PSUM") as tps_pool,
            tc.tile_pool(name="hps", bufs=4, space="PSUM") as hps_pool,
        ):
            cv0 = canv_pool.tile([P, 2, CH, CW], F16, tag="cv0")
            cv1 = canv_pool.tile([P, 2, CH, CW], F16, tag="cv1")
            tbt = const_pool.tile([P, 24], F32, tag="tb")
            hbt = const_pool.tile([P, 3, NBLK], F32, tag="hb")
            nc.sync.dma_start(cv0[:], feat[0])
            nc.sync.dma_start(cv1[:], feat[1])
            nc.sync.dma_start(tbt[:], tb)
            nc.sync.dma_start(hbt[:], hb)

            def emit_conv_layer(rects, layer, cv_in, cv_out, twt):
                for (r0, c0, h, w, slot, _lvl, shrink) in rects:
                    rows = h - 1 - layer if shrink else h
                    for cob in range(2):
                        bias = tbt[:, slot * 8 + layer * 2 + cob][:, None]
                        for (tr, nr) in trunk_tiles(rows, w):
                            ps = tps_pool.tile([P, 480], F32, tag="tps")
                            ps = ps[:, : nr * w]
                            k = 0
                            for dy in (-1, 0, 1):
                                for dx in (-1, 0, 1):
                                    for cib in range(2):
                                        rhs = cv_in[
                                            :, cib,
                                            r0 + tr + dy: r0 + tr + dy + nr,
                                            c0 + dx: c0 + dx + w,
                                        ]
                                        nc.tensor.matmul(
                                            ps,
                                            twt[:, slot, cib, cob, k, :],
                                            rhs,
                                            start=(k == 0 and cib == 0),
                                            stop=(k == 8 and cib == 1),
                                        )
                                    k += 1
                            nc.scalar.activation(
                                cv_out[:, cob, r0 + tr: r0 + tr + nr, c0: c0 + w],
                                ps,
                                mybir.ActivationFunctionType.Relu,
                                bias=bias,
                            )

            def emit_head_rect(rect, pos_base, cv, hwt, nblocks, decode):
                (r0, c0, h, w, slot, _lvl, _shrink) = rect
                hh, _ = rect_head_hw(rect)
                for (tr, nr) in trunk_tiles(hh, w):
                    p0 = pos_base + tr * w
                    n = nr * w
                    for cb in range(nblocks):
                        ps = hps_pool.tile([P, 480], F32, tag="hps")
                        ps = ps[:, :n]
                        k = 0
                        for dy in (-1, 0, 1):
                            for dx in (-1, 0, 1):
                                for cib in range(2):
                                    rhs = cv[
                                        :, cib,
                                        r0 + tr + dy: r0 + tr + dy + nr,
                                        c0 + dx: c0 + dx + w,
                                    ]
                                    nc.tensor.matmul(
                                        ps,
                                        hwt[:, cib, cb, k, :],
                                        rhs,
                                        start=(k == 0 and cib == 0),
                                        stop=(k == 8 and cib == 1),
                                    )
                                k += 1
                        st = st_pool.tile([P, 480], F32, tag="st")
                        st = st[:, :n]
                        nc.scalar.activation(
                            st,
                            ps,
                            mybir.ActivationFunctionType.Identity,
                            bias=hbt[:, slot, cb][:, None],
                        )
                        nc.sync.dma_start(head_out[cb, :, p0: p0 + n], st)
                        if decode and cb < 3:
                            wht = whx_pool.tile([P, 480], F32, tag="wh")
                            cxt = whx_pool.tile([P, 480], F32, tag="cx")
                            nc.sync.dma_start(wht[:, :n], whm[cb, :, p0: p0 + n])
                            nc.sync.dma_start(cxt[:, :n], cxm[cb, :, p0: p0 + n])
                            kst = kst_pool.tile([P, 480], F32, tag="kst")
                            kst = kst[:, :n]
                            nc.vector.tensor_tensor(
                                kst, st, wht[:, :n], mybir.AluOpType.mult)
                            nc.vector.tensor_tensor(
                                kst, kst, cxt[:, :n], mybir.AluOpType.add)
                            nc.sync.dma_start(kpts_out[cb, :, p0: p0 + n], kst)

            def emit_branch(branch):
                # head plan: rect index -> (nblocks, decode), grouped by slot
                if branch == "A1":
                    rects, bases = RECTS_A, POS_BASE_A
                    nslot, plan = 2, {0: (NBLK, False), 1: (1, False)}
                elif branch == "A2":
                    rects, bases = RECTS_A, POS_BASE_A
                    nslot, plan = 2, {0: (3, True), 1: (3, True)}
                else:
                    rects, bases = RECTS_B, POS_BASE_B
                    nslot = 3
                    plan = {g: (HEAD_BLOCKS[SUBNETS[g]], SUBNETS[g] == "kpt")
                            for g in range(3)}
                cvs = [cv0, cv1]
                for layer in range(4):
                    twt = tw_pool.tile([P, 3, 2, 2, 9, P], F16, tag="tw")
                    nc.sync.dma_start(twt[:, 0:nslot], tw[layer][:, 0:nslot])
                    emit_conv_layer(rects, layer, cvs[layer % 2],
                                    cvs[(layer + 1) % 2], twt)
                for g in range(nslot):
                    nblocks, decode = plan[g]
                    hwt = hw_pool.tile([P, 2, NBLK, 9, P], F16, tag="hw")
                    nc.sync.dma_start(hwt[:], hw[g])
                    for ri, rect in enumerate(rects):
                        if rect[4] != g:
                            continue
                        emit_head_rect(rect, bases[ri], cv0, hwt, nblocks, decode)

            pid = nc.partition_id()
            with tc.If(pid < 4) as c1:
                emit_branch("A1")
            with c1.Else():
                with tc.If(pid < 6) as c2:
                    emit_branch("A2")
                with c2.Else():
                    emit_branch("B")

    nc.compile()
    return nc


# ------------------------------------------------------------ host-side prep
def _pack_trunk_w(w4, flip):
    """(4,256,256,3,3) f32 -> (4,128,4608) f16 lhsT slab (cib,cob,tap,coi)."""
    if flip:
        w4 = w4[:, :, :, ::-1, :]
    y = w4.reshape(4, 2, P, 2, P, 3, 3)  # l, cob, coi, cib, cii, dy, dx
    x = np.transpose(y, (0, 4, 3, 1, 5, 6, 2))  # l, cii, cib, cob, dy, dx, coi
    return np.ascontiguousarray(x.reshape(4, P, 2 * 2 * 9 * P)).astype(np.float16)


def _pack_head_w(wh_raw, flip):
    """(Cout,256,3,3) f32 -> (128, 2, NBLK, 9, 128) f16 lhsT, cout padded."""
    if flip:
        wh_raw = wh_raw[:, :, ::-1, :]
    cout = wh_raw.shape[0]
    z = np.zeros((NBLK * P, 256, 3, 3), np.float32)
    z[:cout] = wh_raw
    y = z.reshape(NBLK, P, 2, P, 3, 3).reshape(NBLK, P, 2, P, 9)
    t = np.transpose(y, (3, 2, 0, 4, 1))  # cii, cib, cb, tap, coi
    return np.ascontiguousarray(t).astype(np.float16)


def _feat_to_canvas_block(f):
    """(256,h,w) f32 -> (128,2,h,w) f16."""
    return np.transpose(
        f.reshape(2, P, f.shape[1], f.shape[2]), (1, 0, 2, 3)
    ).astype(np.float16)


def _core_plan(core):
    """Returns (branch, jobs) with jobs[slot] = dict(sub=..., img=..., rect=...,
    base=..., level=..., half=None|'top'|'bot')."""

    def job(slot, sub, img, level, half=None):
        rects = RECTS_A if core < 6 else RECTS_B
        bases = POS_BASE_A if core < 6 else POS_BASE_B
        ris = [i for i, r in enumerate(rects) if r[4] == slot]
        return [
            dict(slot=slot, sub=sub, img=img, level=level, half=half,
                 rect=rects[ri], base=bases[ri])
            for ri in ris
        ]

    if core < 4:  # A1: cls + bbox halves
        img = core // 2
        half = "top" if core % 2 == 0 else "bot"
        return "A1", job(0, "cls", img, 0, half) + job(1, "bbox", img, 0, half)
    if core < 6:  # A2: kpt top + kpt bottom of one image
        img = core - 4
        return "A2", job(0, "kpt", img, 0, "top") + job(1, "kpt", img, 0, "bot")
    img = core - 6
    jobs = []
    for slot, sub in enumerate(SUBNETS):
        for lvl in (1, 2, 3, 4):
            jobs += [j for j in job(slot, sub, img, lvl) if j["rect"][5] == lvl]
    return "B", jobs


def _job_rowbase(j):
    """(npos,) global anchor-row base for a job's head positions."""
    hh, ww = rect_head_hw(j["rect"])
    n = hh * ww
    r, c = np.divmod(np.arange(n), ww)
    if j["half"] == "bot":
        maprow = 79 - r
    else:
        maprow = r
    W = LEVEL_HW[j["level"]]
    return j["img"] * ROWS_PER_IMG + _OFF[j["level"]] + (maprow * W + c) * A


def kernel(**inputs):
    inputs = {k: np.asarray(v) for k, v in inputs.items()}
    from concourse import bass_utils
    from concourse.bass_interp import get_hw_module

    if "nc" not in _PROG_CACHE:
        nc = _build_program()
        nc.m = get_hw_module(nc.m)
        _PROG_CACHE["nc"] = nc
    nc = _PROG_CACHE["nc"]

    feats = [inputs[f"feat{i}"] for i in range(5)]
    boxes = inputs["boxes"].astype(np.float32)
    bw = boxes[:, 2] - boxes[:, 0]
    bh = boxes[:, 3] - boxes[:, 1]
    bcx = boxes[:, 0] + 0.5 * bw
    bcy = boxes[:, 1] + 0.5 * bh

    wkeys = {"cls": ("cls_w", "cls_b", "cls_score_w", "cls_score_b"),
             "bbox": ("bbox_w", "bbox_b", "bbox_pred_w", "bbox_pred_b"),
             "kpt": ("kpt_w", "kpt_b", "kpt_pred_w", "kpt_pred_b")}
    tw_c, hw_c = {}, {}

    def trunk_pack(sub, flip):
        if (sub, flip) not in tw_c:
            tw_c[(sub, flip)] = _pack_trunk_w(
                inputs[wkeys[sub][0]].astype(np.float32), flip)
        return tw_c[(sub, flip)]

    def head_pack(sub, flip):
        if (sub, flip) not in hw_c:
            hw_c[(sub, flip)] = _pack_head_w(
                inputs[wkeys[sub][2]].astype(np.float32), flip)
        return hw_c[(sub, flip)]

    # channel-row decomposition for the kpt decode: channel c = a*34 + k*2 + xy
    c_all = np.arange(3 * P)
    c_a = c_all // 34
    c_xy = c_all % 2
    c_valid = c_all < A * KPT * 2  # 306

    in_maps = []
    core_jobs = []
    for core in range(8):
        branch, jobs = _core_plan(core)
        core_jobs.append(jobs)

        feat_arr = np.zeros((2, P, 2, CH, CW), np.float16)
        tw_arr = np.zeros((4, P, 3, 4608), np.float16)
        tb_arr = np.zeros((P, 24), np.float32)
        hw_arr = np.zeros((3, P, 2, NBLK, 9, P), np.float16)
        hb_arr = np.zeros((P, 3, NBLK), np.float32)
        wh_arr = np.zeros((3, P, TOTPOS), np.float32)
        cx_arr = np.zeros((3, P, TOTPOS), np.float32)

        for j in jobs:
            (r0, c0, h, w, slot, lvl, shrink) = j["rect"]
            sub, img = j["sub"], j["img"]
            flip = j["half"] == "bot"

            fmap = np.asarray(feats[lvl][img], np.float32)
            if j["half"] is not None:
                fmap = fmap[:, ::-1, :] if flip else fmap
                fmap = fmap[:, :HIN, :]  # input rows incl. halo
            feat_arr[0, :, :, r0: r0 + h, c0: c0 + w] = _feat_to_canvas_block(fmap)

            tw_arr[:, :, slot, :] = trunk_pack(sub, flip)
            hw_arr[slot] = head_pack(sub, flip)
            hbv = np.zeros(NBLK * P, np.float32)
            hbv[: HEAD_CH[sub]] = inputs[wkeys[sub][3]].astype(np.float32)
            hb_arr[:, slot, :] = hbv.reshape(NBLK, P).T
            tbm = inputs[wkeys[sub][1]].astype(np.float32)  # (4, 256)
            for layer in range(4):
                for cob in range(2):
                    tb_arr[:, slot * 8 + layer * 2 + cob] = tbm[
                        layer, cob * P: (cob + 1) * P]

            if sub == "kpt":
                rb = _job_rowbase(j)
                n = len(rb)
                rows = rb[None, :] + c_a[c_valid][:, None]  # (306, n)
                whv = np.where(c_xy[c_valid][:, None] == 0, bw[rows], bh[rows])
                cxv = np.where(c_xy[c_valid][:, None] == 0, bcx[rows], bcy[rows])
                wh_arr.reshape(3 * P, TOTPOS)[: 306, j["base"]: j["base"] + n] = whv
                cx_arr.reshape(3 * P, TOTPOS)[: 306, j["base"]: j["base"] + n] = cxv

        in_maps.append({
            "feat": feat_arr,
            "tw": np.ascontiguousarray(tw_arr.reshape(4, P, 3, 2, 2, 9, P)),
            "tb": tb_arr,
            "hw": hw_arr,
            "hb": hb_arr,
            "whm": wh_arr,
            "cxm": cx_arr,
        })

    trace = os.environ.get("CRP_TRACE") == "1"
    kwargs = {}
    if trace:
        _install_ntff_shim()
        kwargs = dict(trace=True, trace_cores=[0, 4, 6])
    res = bass_utils.run_bass_kernel_spmd(
        nc, in_maps, core_ids=list(range(8)), **kwargs)
    globals()["LAST_EXEC_NS"] = res.exec_time_ns

    # ---------------------------------------------------------------- gather
    n_rows = N_IMG * ROWS_PER_IMG
    box_cls = np.zeros((n_rows, C_CLS), np.float32)
    box_delta = np.zeros((n_rows, 4), np.float32)
    pred_kpts = np.zeros((n_rows, KPT, 3), np.float32)
    pred_kpts[:, :, 2] = 1.0

    for core in range(8):
        branch, jobs = _core_plan(core)
        ho = res.results[core]["head_out"].reshape(NBLK * P, TOTPOS)
        ko = res.results[core]["kpts_out"].reshape(3 * P, TOTPOS)
        for j in jobs:
            sub = j["sub"]
            hh, ww = rect_head_hw(j["rect"])
            n = hh * ww
            sl = slice(j["base"], j["base"] + n)
            rb = _job_rowbase(j)
            rows = (rb[:, None] + np.arange(A)[None, :]).ravel()
            if sub == "cls":
                arr = ho[:720, sl].reshape(A, C_CLS, n)
                box_cls[rows] = np.transpose(arr, (2, 0, 1)).reshape(n * A, C_CLS)
            elif sub == "bbox":
                arr = ho[:36, sl].reshape(A, 4, n)
                box_delta[rows] = np.transpose(arr, (2, 0, 1)).reshape(n * A, 4)
            else:
                arr = ko[: A * KPT * 2, sl].reshape(A, KPT, 2, n)
                pred_kpts[rows, :, 0:2] = np.transpose(
                    arr, (3, 0, 1, 2)).reshape(n * A, KPT, 2)

    return box_cls, box_delta, pred_kpts


# -------------------------------------------------- optional NTFF trace shim
def _install_ntff_shim():
    try:
        import antenv

        mod = sys.modules.get("antenv.axon_hooks")
        if mod is None or not hasattr(mod, "get_axon_ntff_profile_hook"):
            mod = types.ModuleType("antenv.axon_hooks")
            mod._hook = None
            mod.set_axon_ntff_profile_hook = lambda h: setattr(mod, "_hook", h)
            mod.get_axon_ntff_profile_hook = lambda: mod._hook
            sys.modules["antenv.axon_hooks"] = mod
            antenv.axon_hooks = mod
        from trn_agent_boot.trn_boot import _ntff_profile_via_ctypes

        hook = _ntff_profile_via_ctypes("/opt/axon/libaxon_pjrt.so")
        if hook is not None:
            mod.set_axon_ntff_profile_hook(hook)
    except Exception:
        pass


# revision 12
# speedup vs baseline: 1.1324x; 1.0357x over previous
"""Trainium2 Bass kernel for CRPNet head (nms_detection problem).

Workload: 5 FPN feature levels x 2 images, three conv subnets (cls/bbox/kpt),
each 4x [conv3x3(256->256) + bias + relu] followed by a conv3x3 head
(cls: 720ch, bbox: 36ch, kpt: 306ch), flattened to per-anchor predictions,
plus the Box2KptTransform keypoint decode.

Strategy (8 NeuronCores, SPMD single program, partition_id branches):
  - cores 0..3 (branch A1): cls + bbox half-maps (level-0 rows split in two,
    5-row shrinking halo; bottom halves are vertically flipped on the host —
    flipped features + dy-flipped weights — so both halves share one geometry).
  - cores 4..5 (branch A2): two kpt half-maps (+ keypoint decode).
  - cores 6..7 (branch B): all level 1-4 maps for one image, all 3 subnets.
Convs are emulated as 9 accumulated shifted matmuls over zero-separated
"canvas" rects in SBUF (SAME padding comes from the zero borders, which stay
zero because separators are never written). All conv matmuls are
channel-major (weights stationary, one free dim; shifted canvas windows are
the moving operand — the stationary operand must be 1-D free on walrus).
Head outputs and the keypoint decode stay channel-major on device; the host
transposes when scattering into the final outputs. Matmul operands are fp16
(PSUM accumulates fp32); per-matmul relative error ~3e-4.
"""

import os
import sys
import types

import numpy as np

# ----------------------------------------------------------------- constants
P = 128
CH, CW = 94, 86  # canvas incl. zero borders/separators
LEVEL_HW = [80, 40, 20, 10, 5]
N_IMG = 2
A = 9
KPT = 17
C_CLS = 80
NBLK = 6  # head cout blocks of 128 (6*128 = 768 >= 720)
TOTPOS = 6400  # per-core head position slots (A: 6400, B: 6375)
HALF = 40  # L0 half-map target rows
HIN = 45  # L0 half-map input rows (target + 5 halo)

SUBNETS = ("cls", "bbox", "kpt")
HEAD_CH = {"cls": 720, "bbox": 36, "kpt": 306}
HEAD_BLOCKS = {"cls": 6, "bbox": 1, "kpt": 3}

_HWA = [A * h * h for h in LEVEL_HW]
_OFF = [0]
for _s in _HWA:
    _OFF.append(_OFF[-1] + _s)
ROWS_PER_IMG = _OFF[-1]  # 76725

# rects: (r0, c0, h_in, w, slot, level, shrink)
#   shrink=True: conv layer l writes local rows [0, h_in-1-l); head writes
#   [0, h_in-5). shrink=False: every layer writes all h_in rows.
RECTS_A = [
    (1, 1, HIN, 80, 0, 0, True),
    (47, 1, HIN, 80, 1, 0, True),
]
RECTS_B = [
    (1, 1, 40, 40, 0, 1, False), (1, 42, 40, 40, 1, 1, False),
    (42, 1, 40, 40, 2, 1, False),
    (42, 42, 20, 20, 0, 2, False), (42, 63, 20, 20, 1, 2, False),
    (63, 42, 20, 20, 2, 2, False),
    (63, 63, 10, 10, 0, 3, False), (63, 74, 10, 10, 1, 3, False),
    (74, 63, 10, 10, 2, 3, False),
    (74, 74, 5, 5, 0, 4, False), (74, 80, 5, 5, 1, 4, False),
    (80, 74, 5, 5, 2, 4, False),
]


def rect_head_hw(rect):
    (_r0, _c0, h, w, _s, _l, shrink) = rect
    return (h - 5, w) if shrink else (h, w)


def rect_pos_bases(rects):
    bases, off = [], 0
    for rect in rects:
        bases.append(off)
        hh, ww = rect_head_hw(rect)
        off += hh * ww
    assert off <= TOTPOS
    return bases


POS_BASE_A = rect_pos_bases(RECTS_A)
POS_BASE_B = rect_pos_bases(RECTS_B)


def trunk_tiles(h, w):
    """Row-strips of <=512 cells: list of (row_start, n_rows)."""
    rpt = min(h, 512 // w)
    out = []
    r = 0
    while r < h:
        n = min(rpt, h - r)
        out.append((r, n))
        r += n
    return out


# ------------------------------------------------------------- program build
_PROG_CACHE = {}


def _build_program():
    from concourse import bacc, mybir
    import concourse.tile as tile

    F16 = mybir.dt.float16
    F32 = mybir.dt.float32

    nc = bacc.Bacc(
        "TRN2",
        target_bir_lowering=False,
        debug=False,
        enable_asserts=False,
        num_devices=8,
    )

    feat = nc.dram_tensor("feat", [P, 2, CH, CW], F16, kind="ExternalInput").ap()
    tw = nc.dram_tensor("tw", [4, P, 3, 2, 2, 9, P], F16, kind="ExternalInput").ap()
    tb = nc.dram_tensor("tb", [P, 24], F32, kind="ExternalInput").ap()
    hw = nc.dram_tensor("hw", [3, P, 2, NBLK, 9, P], F16, kind="ExternalInput").ap()
    hb = nc.dram_tensor("hb", [P, 3, NBLK], F32, kind="ExternalInput").ap()
    whm = nc.dram_tensor("whm", [3, P, TOTPOS], F32, kind="ExternalInput").ap()
    cxm = nc.dram_tensor("cxm", [3, P, TOTPOS], F32, kind="ExternalInput").ap()
    head_out = nc.dram_tensor(
        "head_out", [NBLK, P, TOTPOS], F32, kind="ExternalOutput").ap()
    kpts_out = nc.dram_tensor(
        "kpts_out", [3, P, TOTPOS], F32, kind="ExternalOutput").ap()

    with tile.TileContext(nc) as tc:
        with (
            tc.tile_pool(name="canv", bufs=1) as canv_pool,
            tc.tile_pool(name="tw", bufs=2) as tw_pool,
            tc.tile_pool(name="hw", bufs=2) as hw_pool,
            tc.tile_pool(name="const", bufs=1) as const_pool,
            tc.tile_pool(name="st", bufs=4) as st_pool,
            tc.tile_pool(name="kst", bufs=3) as kst_pool,
            tc.tile_pool(name="whx", bufs=3) as whx_pool,
            tc.tile_pool(name="tps", bufs=4, space="PSUM") as tps_pool,
            tc.tile_pool(name="hps", bufs=4, space="PSUM") as hps_pool,
        ):
            # load partition_id before any bulk DMA is enqueued: every engine's
            # branch waits on this tiny load, and the per-branch weight DMAs
            # can't issue until the sync engine has branched.
            pid = nc.partition_id()

            cv0 = canv_pool.tile([P, 2, CH, CW], F16, tag="cv0")
            cv1 = canv_pool.tile([P, 2, CH, CW], F16, tag="cv1")
            tbt = const_pool.tile([P, 24], F32, tag="tb")
            hbt = const_pool.tile([P, 3, NBLK], F32, tag="hb")
            nc.sync.dma_start(tbt[:], tb)
            nc.sync.dma_start(hbt[:], hb)
            nc.sync.dma_start(cv0[:], feat)
            nc.gpsimd.memset(cv1[:], 0.0)

            def emit_conv_layer(rects, layer, cv_in, cv_out, twts):
                for (r0, c0, h, w, slot, _lvl, shrink) in rects:
                    rows = h - 1 - layer if shrink else h
                    twt = twts[slot]
                    for cob in range(2):
                        bias = tbt[:, slot * 8 + layer * 2 + cob][:, None]
                        for (tr, nr) in trunk_tiles(rows, w):
                            ps = tps_pool.tile([P, 480], F32, tag="tps")
                            ps = ps[:, : nr * w]
                            k = 0
                            for dy in (-1, 0, 1):
                                for dx in (-1, 0, 1):
                                    for cib in range(2):
                                        rhs = cv_in[
                                            :, cib,
                                            r0 + tr + dy: r0 + tr + dy + nr,
                                            c0 + dx: c0 + dx + w,
                                        ]
                                        nc.tensor.matmul(
                                            ps,
                                            twt[:, 0, cib, cob, k, :],
                                            rhs,
                                            start=(k == 0 and cib == 0),
                                            stop=(k == 8 and cib == 1),
                                        )
                                    k += 1
                            nc.scalar.activation(
                                cv_out[:, cob, r0 + tr: r0 + tr + nr, c0: c0 + w],
                                ps,
                                mybir.ActivationFunctionType.Relu,
                                bias=bias,
                            )

            def emit_head_rect(rect, pos_base, cv, hwt, nblocks, decode):
                (r0, c0, h, w, slot, _lvl, _shrink) = rect
                hh, _ = rect_head_hw(rect)
                for (tr, nr) in trunk_tiles(hh, w):
                    p0 = pos_base + tr * w
                    n = nr * w
                    for cb in range(nblocks):
                        ps = hps_pool.tile([P, 480], F32, tag="hps")
                        ps = ps[:, :n]
                        k = 0
                        for dy in (-1, 0, 1):
                            for dx in (-1, 0, 1):
                                for cib in range(2):
                                    rhs = cv[
                                        :, cib,
                                        r0 + tr + dy: r0 + tr + dy + nr,
                                        c0 + dx: c0 + dx + w,
                                    ]
                                    nc.tensor.matmul(
                                        ps,
                                        hwt[:, cib, cb, k, :],
                                        rhs,
                                        start=(k == 0 and cib == 0),
                                        stop=(k == 8 and cib == 1),
                                    )
                                k += 1
                        st = st_pool.tile([P, 480], F32, tag="st")
                        st = st[:, :n]
                        nc.scalar.activation(
                            st,
                            ps,
                            mybir.ActivationFunctionType.Identity,
                            bias=hbt[:, slot, cb][:, None],
                        )
                        nc.sync.dma_start(head_out[cb, :, p0: p0 + n], st)
                        if decode and cb < 3:
                            wht = whx_pool.tile([P, 480], F32, tag="wh")
                            cxt = whx_pool.tile([P, 480], F32, tag="cx")
                            nc.sync.dma_start(wht[:, :n], whm[cb, :, p0: p0 + n])
                            nc.sync.dma_start(cxt[:, :n], cxm[cb, :, p0: p0 + n])
                            kst = kst_pool.tile([P, 480], F32, tag="kst")
                            kst = kst[:, :n]
                            nc.vector.tensor_tensor(
                                kst, st, wht[:, :n], mybir.AluOpType.mult)
                            nc.vector.tensor_tensor(
                                kst, kst, cxt[:, :n], mybir.AluOpType.add)
                            nc.sync.dma_start(kpts_out[cb, :, p0: p0 + n], kst)

            def emit_branch(branch):
                # head plan: rect index -> (nblocks, decode), grouped by slot
                if branch == "A1":
                    rects, bases = RECTS_A, POS_BASE_A
                    nslot, plan = 2, {0: (NBLK, False), 1: (1, False)}
                elif branch == "A2":
                    rects, bases = RECTS_A, POS_BASE_A
                    nslot, plan = 2, {0: (3, True), 1: (3, True)}
                else:
                    rects, bases = RECTS_B, POS_BASE_B
                    nslot = 3
                    plan = {g: (HEAD_BLOCKS[SUBNETS[g]], SUBNETS[g] == "kpt")
                            for g in range(3)}
                # slot-major rect order: slot-s tiles only wait on slot-s weights
                pairs = sorted(zip(rects, bases), key=lambda rb: rb[0][4])
                rects = [rb[0] for rb in pairs]
                bases = [rb[1] for rb in pairs]
                cvs = [cv0, cv1]
                for layer in range(4):
                    twts = []
                    for s in range(nslot):
                        twt = tw_pool.tile([P, 1, 2, 2, 9, P], F16, tag=f"tw{s}")
                        nc.sync.dma_start(twt[:], tw[layer][:, s: s + 1])
                        twts.append(twt)
                    emit_conv_layer(rects, layer, cvs[layer % 2],
                                    cvs[(layer + 1) % 2], twts)
                for g in range(nslot):
                    nblocks, decode = plan[g]
                    hwt = hw_pool.tile([P, 2, NBLK, 9, P], F16, tag="hw")
                    nc.sync.dma_start(hwt[:], hw[g])
                    for ri, rect in enumerate(rects):
                        if rect[4] != g:
                            continue
                        emit_head_rect(rect, bases[ri], cv0, hwt, nblocks, decode)

            pid = nc.partition_id()
            with tc.If(pid < 4) as c1:
                emit_branch("A1")
            with c1.Else():
                with tc.If(pid < 6) as c2:
                    emit_branch("A2")
                with c2.Else():
                    emit_branch("B")

    nc.compile()
    return nc


# ------------------------------------------------------------ host-side prep
def _pack_trunk_w(w4, flip):
    """(4,256,256,3,3) f32 -> (4,128,4608) f16 lhsT slab (cib,cob,tap,coi)."""
    if flip:
        w4 = w4[:, :, :, ::-1, :]
    y = w4.reshape(4, 2, P, 2, P, 3, 3)  # l, cob, coi, cib, cii, dy, dx
    x = np.transpose(y, (0, 4, 3, 1, 5, 6, 2))  # l, cii, cib, cob, dy, dx, coi
    return np.ascontiguousarray(x.reshape(4, P, 2 * 2 * 9 * P)).astype(np.float16)


def _pack_head_w(wh_raw, flip):
    """(Cout,256,3,3) f32 -> (128, 2, NBLK, 9, 128) f16 lhsT, cout padded."""
    if flip:
        wh_raw = wh_raw[:, :, ::-1, :]
    cout = wh_raw.shape[0]
    z = np.zeros((NBLK * P, 256, 3, 3), np.float32)
    z[:cout] = wh_raw
    y = z.reshape(NBLK, P, 2, P, 3, 3).reshape(NBLK, P, 2, P, 9)
    t = np.transpose(y, (3, 2, 0, 4, 1))  # cii, cib, cb, tap, coi
    return np.ascontiguousarray(t).astype(np.float16)


def _feat_to_canvas_block(f):
    """(256,h,w) f32 -> (128,2,h,w) f16."""
    return np.transpose(
        f.reshape(2, P, f.shape[1], f.shape[2]), (1, 0, 2, 3)
    ).astype(np.float16)


def _core_plan(core):
    """Returns (branch, jobs) with jobs[slot] = dict(sub=..., img=..., rect=...,
    base=..., level=..., half=None|'top'|'bot')."""

    def job(slot, sub, img, level, half=None):
        rects = RECTS_A if core < 6 else RECTS_B
        bases = POS_BASE_A if core < 6 else POS_BASE_B
        ris = [i for i, r in enumerate(rects) if r[4] == slot]
        return [
            dict(slot=slot, sub=sub, img=img, level=level, half=half,
                 rect=rects[ri], base=bases[ri])
            for ri in ris
        ]

    if core < 4:  # A1: cls + bbox halves
        img = core // 2
        half = "top" if core % 2 == 0 else "bot"
        return "A1", job(0, "cls", img, 0, half) + job(1, "bbox", img, 0, half)
    if core < 6:  # A2: kpt top + kpt bottom of one image
        img = core - 4
        return "A2", job(0, "kpt", img, 0, "top") + job(1, "kpt", img, 0, "bot")
    img = core - 6
    jobs = []
    for slot, sub in enumerate(SUBNETS):
        for lvl in (1, 2, 3, 4):
            jobs += [j for j in job(slot, sub, img, lvl) if j["rect"][5] == lvl]
    return "B", jobs


def _job_rowbase(j):
    """(npos,) global anchor-row base for a job's head positions."""
    hh, ww = rect_head_hw(j["rect"])
    n = hh * ww
    r, c = np.divmod(np.arange(n), ww)
    if j["half"] == "bot":
        maprow = 79 - r
    else:
        maprow = r
    W = LEVEL_HW[j["level"]]
    return j["img"] * ROWS_PER_IMG + _OFF[j["level"]] + (maprow * W + c) * A


def kernel(**inputs):
    inputs = {k: np.asarray(v) for k, v in inputs.items()}
    from concourse import bass_utils
    from concourse.bass_interp import get_hw_module

    if "nc" not in _PROG_CACHE:
        nc = _build_program()
        nc.m = get_hw_module(nc.m)
        _PROG_CACHE["nc"] = nc
    nc = _PROG_CACHE["nc"]

    feats = [inputs[f"feat{i}"] for i in range(5)]
    boxes = inputs["boxes"].astype(np.float32)
    bw = boxes[:, 2] - boxes[:, 0]
    bh = boxes[:, 3] - boxes[:, 1]
    bcx = boxes[:, 0] + 0.5 * bw
    bcy = boxes[:, 1] + 0.5 * bh

    wkeys = {"cls": ("cls_w", "cls_b", "cls_score_w", "cls_score_b"),
             "bbox": ("bbox_w", "bbox_b", "bbox_pred_w", "bbox_pred_b"),
             "kpt": ("kpt_w", "kpt_b", "kpt_pred_w", "kpt_pred_b")}
    tw_c, hw_c = {}, {}

    def trunk_pack(sub, flip):
        if (sub, flip) not in tw_c:
            tw_c[(sub, flip)] = _pack_trunk_w(
                inputs[wkeys[sub][0]].astype(np.float32), flip)
        return tw_c[(sub, flip)]

    def head_pack(sub, flip):
        if (sub, flip) not in hw_c:
            hw_c[(sub, flip)] = _pack_head_w(
                inputs[wkeys[sub][2]].astype(np.float32), flip)
        return hw_c[(sub, flip)]

    # channel-row decomposition for the kpt decode: channel c = a*34 + k*2 + xy
    c_all = np.arange(3 * P)
    c_a = c_all // 34
    c_xy = c_all % 2
    c_valid = c_all < A * KPT * 2  # 306

    in_maps = []
    core_jobs = []
    for core in range(8):
        branch, jobs = _core_plan(core)
        core_jobs.append(jobs)

        feat_arr = np.zeros((P, 2, CH, CW), np.float16)
        tw_arr = np.zeros((4, P, 3, 4608), np.float16)
        tb_arr = np.zeros((P, 24), np.float32)
        hw_arr = np.zeros((3, P, 2, NBLK, 9, P), np.float16)
        hb_arr = np.zeros((P, 3, NBLK), np.float32)
        wh_arr = np.zeros((3, P, TOTPOS), np.float32)
        cx_arr = np.zeros((3, P, TOTPOS), np.float32)

        for j in jobs:
            (r0, c0, h, w, slot, lvl, shrink) = j["rect"]
            sub, img = j["sub"], j["img"]
            flip = j["half"] == "bot"

            fmap = np.asarray(feats[lvl][img], np.float32)
            if j["half"] is not None:
                fmap = fmap[:, ::-1, :] if flip else fmap
                fmap = fmap[:, :HIN, :]  # input rows incl. halo
            feat_arr[:, :, r0: r0 + h, c0: c0 + w] = _feat_to_canvas_block(fmap)

            tw_arr[:, :, slot, :] = trunk_pack(sub, flip)
            hw_arr[slot] = head_pack(sub, flip)
            hbv = np.zeros(NBLK * P, np.float32)
            hbv[: HEAD_CH[sub]] = inputs[wkeys[sub][3]].astype(np.float32)
            hb_arr[:, slot, :] = hbv.reshape(NBLK, P).T
            tbm = inputs[wkeys[sub][1]].astype(np.float32)  # (4, 256)
            for layer in range(4):
                for cob in range(2):
                    tb_arr[:, slot * 8 + layer * 2 + cob] = tbm[
                        layer, cob * P: (cob + 1) * P]

            if sub == "kpt":
                rb = _job_rowbase(j)
                n = len(rb)
                rows = rb[None, :] + c_a[c_valid][:, None]  # (306, n)
                whv = np.where(c_xy[c_valid][:, None] == 0, bw[rows], bh[rows])
                cxv = np.where(c_xy[c_valid][:, None] == 0, bcx[rows], bcy[rows])
                wh_arr.reshape(3 * P, TOTPOS)[: 306, j["base"]: j["base"] + n] = whv
                cx_arr.reshape(3 * P, TOTPOS)[: 306, j["base"]: j["base"] + n] = cxv

        in_maps.append({
            "feat": feat_arr,
            "tw": np.ascontiguousarray(tw_arr.reshape(4, P, 3, 2, 2, 9, P)),
            "tb": tb_arr,
            "hw": hw_arr,
            "hb": hb_arr,
            "whm": wh_arr,
            "cxm": cx_arr,
        })

    trace = os.environ.get("CRP_TRACE") == "1"
    kwargs = {}
    if trace:
        _install_ntff_shim()
        kwargs = dict(trace=True, trace_cores=[0, 4, 6])
    res = bass_utils.run_bass_kernel_spmd(
        nc, in_maps, core_ids=list(range(8)), **kwargs)
    globals()["LAST_EXEC_NS"] = res.exec_time_ns

    # ---------------------------------------------------------------- gather
    n_rows = N_IMG * ROWS_PER_IMG
    box_cls = np.zeros((n_rows, C_CLS), np.float32)
    box_delta = np.zeros((n_rows, 4), np.float32)
    pred_kpts = np.zeros((n_rows, KPT, 3), np.float32)
    pred_kpts[:, :, 2] = 1.0

    for core in range(8):
        branch, jobs = _core_plan(core)
        ho = res.results[core]["head_out"].reshape(NBLK * P, TOTPOS)
        ko = res.results[core]["kpts_out"].reshape(3 * P, TOTPOS)
        for j in jobs:
            sub = j["sub"]
            hh, ww = rect_head_hw(j["rect"])
            n = hh * ww
            sl = slice(j["base"], j["base"] + n)
            rb = _job_rowbase(j)
            rows = (rb[:, None] + np.arange(A)[None, :]).ravel()
            if sub == "cls":
                arr = ho[:720, sl].reshape(A, C_CLS, n)
                box_cls[rows] = np.transpose(arr, (2, 0, 1)).reshape(n * A, C_CLS)
            elif sub == "bbox":
                arr = ho[:36, sl].reshape(A, 4, n)
                box_delta[rows] = np.transpose(arr, (2, 0, 1)).reshape(n * A, 4)
            else:
                arr = ko[: A * KPT * 2, sl].reshape(A, KPT, 2, n)
                pred_kpts[rows, :, 0:2] = np.transpose(
                    arr, (3, 0, 1, 2)).reshape(n * A, KPT, 2)

    return box_cls, box_delta, pred_kpts


# -------------------------------------------------- optional NTFF trace shim
def _install_ntff_shim():
    try:
        import antenv

        mod = sys.modules.get("antenv.axon_hooks")
        if mod is None or not hasattr(mod, "get_axon_ntff_profile_hook"):
            mod = types.ModuleType("antenv.axon_hooks")
            mod._hook = None
            mod.set_axon_ntff_profile_hook = lambda h: setattr(mod, "_hook", h)
            mod.get_axon_ntff_profile_hook = lambda: mod._hook
            sys.modules["antenv.axon_hooks"] = mod
            antenv.axon_hooks = mod
        from trn_agent_boot.trn_boot import _ntff_profile_via_ctypes

        hook = _ntff_profile_via_ctypes("/opt/axon/libaxon_pjrt.so")
        if hook is not None:
            mod.set_axon_ntff_profile_hook(hook)
    except Exception:
        pass
